# revision 13
# baseline (speedup 1.0000x reference)
"""NearbyAttention Trainium2 kernel.

Full-input contract: kernel(**inputs) takes the unsharded numpy inputs of
nn_NearbyAttention (q,k,v: [16,1025,1024] f32; Wq/Wk/Wv/Wo: [1024,1024] f32;
bo: [1024] f32) and returns the full [16,1025,1024] f32 output.

Strategy: 8-way data parallel over the batch dim (2 batches per NeuronCore),
weights replicated, no collectives. Host pre-transposes activations to
[dim, tokens], folds the 1/sqrt(dh) scale into Wq, and casts to bf16.

Per batch on device:
  - v projection into per-key-chunk head panels with a trailing ones column
    (softmax denominator rides the PV accumulation for free)
  - q/k head-pair projections software-pipelined one head-pair ahead of the
    sparse nearby attention so the PE stream stays dense (HAM stays warm)
  - BOS-as-key scores for both heads of a pair via one masked [128,65]
    stationary matmul + batched exp; BOS-as-query softmax denominator via
    activation accum_out; epilogue folded into the hp loop
  - output projection back to [tokens, 1024] fp32
"""

import os

import numpy as np
import ml_dtypes

import concourse.bass as bass
import concourse.mybir as mybir
import concourse.tile as tile
from concourse import bacc
from concourse.bass_utils import run_bass_kernel_spmd

BF16 = mybir.dt.bfloat16
F32 = mybir.dt.float32
AF = mybir.ActivationFunctionType
ALU = mybir.AluOpType

B = 16              # full batch
BPC = 2             # batches per core
NCORES = 8
NT = 1025           # tokens (BOS + 32*32 grid)
G = 1024            # grid tokens
DIM = 1024
HEADS = 16
DH = 64
INNER = HEADS * DH  # 1024
P = 128
KT = 8              # contraction tiles of 128
SCALE = DH ** -0.5  # 0.125 (folded into Wq on host)
NEG = -30.0         # mask bias for the handful of memset'd cells

KC = 8              # key chunks of 128 grid tokens (4 patch rows each)

HH = (slice(0, 64), slice(64, 128))


def _mask_pattern() -> np.ndarray:
    """[128 key-local, 256 query-local] 0/1 bf16 mask in S^T orientation.

    Key chunk rows kr=0..3 (absolute 4kc+kr); query window rows
    rho=0..7 (absolute 4kc-2+rho). Unmasked iff |kr+2-rho|<=2 and
    |kcol-qcol|<=2.
    """
    kr = np.arange(128)[:, None] // 32
    kcol = np.arange(128)[:, None] % 32
    qr = np.arange(256)[None, :] // 32
    qcol = np.arange(256)[None, :] % 32
    m = (np.abs(kr + 2 - qr) <= 2) & (np.abs(kcol - qcol) <= 2)
    return m.astype(ml_dtypes.bfloat16)


def _qwin(kc: int) -> tuple[int, int, int]:
    """Grid-query window for key chunk kc: (grid_start, width, mask_col_off)."""
    if kc == 0:
        return 0, 192, 64
    if kc == KC - 1:
        return 128 * kc - 64, 192, 0
    return 128 * kc - 64, 256, 0


def build_nc():
    nc = bacc.Bacc("TRN2", target_bir_lowering=False, debug=False,
                   num_devices=NCORES)

    qt = nc.dram_tensor("qt", [BPC, DIM, NT], BF16, kind="ExternalInput")
    kt = nc.dram_tensor("kt", [BPC, DIM, NT], BF16, kind="ExternalInput")
    vt = nc.dram_tensor("vt", [BPC, DIM, NT], BF16, kind="ExternalInput")
    wq = nc.dram_tensor("wq", [DIM, INNER], BF16, kind="ExternalInput")
    wk = nc.dram_tensor("wk", [DIM, INNER], BF16, kind="ExternalInput")
    wv = nc.dram_tensor("wv", [DIM, INNER], BF16, kind="ExternalInput")
    wo = nc.dram_tensor("wo", [INNER, DIM], BF16, kind="ExternalInput")
    out = nc.dram_tensor("out", [BPC, NT, DIM], F32, kind="ExternalOutput")

    mask_np = _mask_pattern()  # [128, 256]
    mask2_np = np.concatenate([mask_np, mask_np], axis=1)  # [128, 512]
    mask_dram = nc.inline_tensor(mask2_np, name="mask2")

    with tile.TileContext(nc) as tc:
        with (
            tc.tile_pool(name="singles", bufs=1) as singles,
            tc.tile_pool(name="perbatch", bufs=1) as perbatch,
            tc.tile_pool(name="hppool", bufs=3) as hppool,
            tc.tile_pool(name="ppool", bufs=3) as ppool,
            tc.tile_pool(name="small", bufs=2) as small,
            tc.tile_pool(name="bcast", bufs=2) as bcast,
            tc.tile_pool(name="psA", bufs=4, space="PSUM") as psA,
            tc.tile_pool(name="psPV", bufs=4, space="PSUM") as psPV,
        ):
            # ---- persistent weights/constants, split per 128-row ktile ----
            wq_c = [singles.tile([P, INNER], BF16, tag=f"wq{c}", name=f"wq{c}")
                    for c in range(KT)]
            wk_c = [singles.tile([P, INNER], BF16, tag=f"wk{c}", name=f"wk{c}")
                    for c in range(KT)]
            wv_c = [singles.tile([P, INNER], BF16, tag=f"wv{c}", name=f"wv{c}")
                    for c in range(KT)]
            wo_c = [singles.tile([P, DIM], BF16, tag=f"wo{c}", name=f"wo{c}")
                    for c in range(KT)]
            mask2_sb = singles.tile([P, 512], BF16, tag="mask2")
            ones_sb = singles.tile([P, 1], F32, tag="ones")
            nc.vector.memset(ones_sb[:], 1.0)

            for b in range(BPC):
                # ---- load transposed activations (per-ktile slices) ----
                vT = [perbatch.tile([P, NT], BF16, tag=f"vT{c}", name=f"vT{c}")
                      for c in range(KT)]
                qT = [perbatch.tile([P, NT], BF16, tag=f"qT{c}", name=f"qT{c}")
                      for c in range(KT)]
                kT = [perbatch.tile([P, NT], BF16, tag=f"kT{c}", name=f"kT{c}")
                      for c in range(KT)]
                for c in range(KT):
                    nc.sync.dma_start(vT[c][:], vt[b, 128 * c: 128 * (c + 1), :])
                    if b == 0:
                        nc.sync.dma_start(wv_c[c][:], wv[128 * c: 128 * (c + 1), :])
                for c in range(KT):
                    nc.sync.dma_start(qT[c][:], qt[b, 128 * c: 128 * (c + 1), :])
                    if b == 0:
                        nc.sync.dma_start(wq_c[c][:], wq[128 * c: 128 * (c + 1), :])
                for c in range(KT):
                    nc.sync.dma_start(kT[c][:], kt[b, 128 * c: 128 * (c + 1), :])
                    if b == 0:
                        nc.sync.dma_start(wk_c[c][:], wk[128 * c: 128 * (c + 1), :])
                if b == 0:
                    nc.sync.dma_start(mask2_sb[:], mask_dram[:])
                    for c in range(KT):
                        nc.sync.dma_start(wo_c[c][:], wo[128 * c: 128 * (c + 1), :])

                # ---- v projection into head panels ----
                # vh_panel[g, kc, h, DH] = 1 (softmax denominator trick, z
                # on PSUM partition DH); [.., 0:DH] = (v@Wv)[token g+1, h*64:]
                vh_panel = perbatch.tile([P, KC, HEADS, DH + 1], BF16, tag="vhp")
                vbos = perbatch.tile([1, HEADS, DH + 1], BF16, tag="vbos")
                vbos65 = perbatch.tile([DH + 1, HEADS, DH + 1], BF16,
                                       tag="vbos65")
                nc.vector.memset(vh_panel[:, :, :, DH], 1.0)
                nc.vector.memset(vbos[:, :, DH], 1.0)

                for mt in range(KC):  # grid token chunks
                    for half in range(2):
                        acc = psA.tile([P, 512], F32, tag="ps")
                        for c in range(KT):
                            nc.tensor.matmul(
                                acc[:],
                                vT[c][:, 1 + 128 * mt: 1 + 128 * (mt + 1)],
                                wv_c[c][:, 512 * half: 512 * (half + 1)],
                                start=(c == 0), stop=(c == KT - 1),
                            )
                        nc.vector.tensor_copy(
                            vh_panel[:, mt, 8 * half: 8 * (half + 1), 0:DH],
                            acc.rearrange("p (h d) -> p h d", d=DH),
                        )
                # BOS token of v
                for half in range(2):
                    acc = psA.tile([P, 512], F32, tag="ps")
                    for c in range(KT):
                        nc.tensor.matmul(
                            acc[0:1, :], vT[c][:, 0:1],
                            wv_c[c][:, 512 * half: 512 * (half + 1)],
                            start=(c == 0), stop=(c == KT - 1),
                        )
                    nc.vector.tensor_copy(
                        vbos[:, 8 * half: 8 * (half + 1), 0:DH],
                        acc[0:1].rearrange("p (h d) -> p h d", d=DH),
                    )
                # replicate the BOS v panel across partitions so the PV
                # start matmul's stationary can be based at partition 0
                # (head 0) or 64 (head 1) to match the pbos65 row
                nc.gpsimd.partition_broadcast(vbos65[:], vbos[0:1])

                attnout = perbatch.tile([P, KC, NT], BF16, tag="attnout")

                # ---- software-pipelined q/k head-pair projections ----
                def make_proj_groups(hp, qhT, khT):
                    """Six emitters: (dst, chunk) psA groups for head pair hp."""
                    groups = []
                    for dst, w_c, src in ((khT, wk_c, kT), (qhT, wq_c, qT)):
                        for nt0, ntw in ((0, 512), (512, 512), (1024, 1)):
                            def g(dst=dst, w_c=w_c, src=src, nt0=nt0, ntw=ntw):
                                acc = psA.tile([P, 512], F32, tag="ps")
                                for c in range(KT):
                                    nc.tensor.matmul(
                                        acc[:, 0:ntw],
                                        w_c[c][:, 128 * hp: 128 * (hp + 1)],
                                        src[c][:, nt0: nt0 + ntw],
                                        start=(c == 0), stop=(c == KT - 1),
                                    )
                                nc.vector.tensor_copy(
                                    dst[:, nt0: nt0 + ntw], acc[:, 0:ntw])
                            groups.append(g)
                    return groups

                hp_tiles = []
                for hp in range(KC):
                    qhT = hppool.tile([P, NT], BF16, tag="qhT",
                                      name=f"qhT{hp}")
                    khT = hppool.tile([P, NT], BF16, tag="khT",
                                      name=f"khT{hp}")
                    hp_tiles.append((qhT, khT))

                pending = list(make_proj_groups(0, *hp_tiles[0]))

                for hp in range(KC):
                    qhT, khT = hp_tiles[hp]
                    for g in pending:  # finish this hp's projections
                        g()
                    pending = (list(make_proj_groups(hp + 1, *hp_tiles[hp + 1]))
                               if hp < KC - 1 else [])

                    def fill(n):
                        for _ in range(min(n, len(pending))):
                            pending.pop(0)()

                    # ---- BOS-as-key scores: head 0 lands on partition 0,
                    # head 1 on partition 64 (legal matmul base partitions)
                    khB = small.tile([P, DH + 1], BF16, tag="khB")
                    nc.vector.memset(khB[:], 0.0)
                    nc.vector.tensor_copy(khB[HH[0], 0:1], khT[HH[0], 0:1])
                    nc.vector.tensor_copy(khB[HH[1], DH: DH + 1],
                                          khT[HH[1], 0:1])
                    pbos65 = hppool.tile([DH + 1, NT], BF16, tag="pbos65")
                    for nt0, ntw in ((0, 512), (512, 512), (1024, 1)):
                        acc = psA.tile([P, 512], F32, tag="ps")
                        nc.tensor.matmul(acc[0:DH + 1, 0:ntw], khB[:],
                                         qhT[:, nt0: nt0 + ntw],
                                         start=True, stop=True)
                        nc.scalar.activation(pbos65[:, nt0: nt0 + ntw],
                                             acc[0:DH + 1, 0:ntw], AF.Exp)
                    fill(2)

                    # ---- BOS-as-query scores; zq via exp accum_out ----
                    pq2 = hppool.tile([P, 2, 9], BF16, tag="pq2")
                    pqsum = small.tile([P, 2], F32, tag="pqsum")
                    sq2 = [psA.tile([P, 512], F32, tag="ps", name=f"sq2_{i}")
                           for i in range(2)]
                    for hh in range(2):
                        nc.vector.memset(sq2[hh][:, 0:1], NEG)
                    for hh in range(2):
                        nc.tensor.matmul(sq2[hh][0:1, 0:1], khT[HH[hh], 0:1],
                                         qhT[HH[hh], 0:1], start=True, stop=True)
                        for c in range(KC):
                            nc.tensor.matmul(
                                sq2[hh][:, 1 + c: 2 + c],
                                khT[HH[hh], 1 + 128 * c: 1 + 128 * (c + 1)],
                                qhT[HH[hh], 0:1], start=True, stop=True,
                            )
                    zq_ps = psA.tile([P, 512], F32, tag="ps")
                    for hh in range(2):
                        nc.scalar.activation(pq2[:, hh, :], sq2[hh][:, 0:9],
                                             AF.Exp,
                                             accum_out=pqsum[:, hh: hh + 1])
                        nc.tensor.matmul(zq_ps[0:1, hh: hh + 1], ones_sb[:],
                                         pqsum[:, hh: hh + 1],
                                         start=True, stop=True)
                    rzq2 = small.tile([1, 2], F32, tag="rzq2")
                    nc.vector.reciprocal_approx_fast(rzq2[:], zq_ps[0:1, 0:2])
                    rzqb = small.tile([P, 2], F32, tag="rzqb")
                    nc.gpsimd.partition_broadcast(rzqb[:], rzq2[:])
                    for hh in range(2):
                        nc.vector.tensor_tensor(
                            pq2[:, hh, :], pq2[:, hh, :],
                            rzqb[:, hh: hh + 1].to_broadcast([P, 9]), ALU.mult)
                    fill(2)

                    # ---- main nearby attention, both heads interleaved ----
                    pv2 = [[psPV.tile([DH + 1, 512], F32, tag="pv",
                                      name=f"pv{i}_{j}") for j in range(2)]
                           for i in range(2)]
                    for hh in range(2):
                        h = 2 * hp + hh
                        hb = DH * hh  # base partition 0 or 64
                        for j in range(2):
                            nc.tensor.matmul(
                                pv2[hh][j][:], vbos65[hb: hb + 1, h, :],
                                pbos65[hb: hb + 1,
                                       1 + 512 * j: 1 + 512 * (j + 1)],
                                start=True, stop=False, skip_group_check=True)

                    for kcp in range(4):  # pairs of key chunks
                        kca, kcb = 2 * kcp, 2 * kcp + 1
                        s2 = [psA.tile([P, 512], F32, tag="ps", name=f"s2_{i}")
                              for i in range(2)]
                        p2 = [ppool.tile([P, 512], BF16, tag="p", name=f"p2_{i}")
                              for i in range(2)]
                        # all four QK matmuls adjacent: h0/h1 use disjoint
                        # PE row groups and run concurrently
                        for j, kc in enumerate((kca, kcb)):
                            g0, w, _ = _qwin(kc)
                            for hh in range(2):
                                nc.tensor.matmul(
                                    s2[hh][:, 256 * j: 256 * j + w],
                                    khT[HH[hh], 1 + 128 * kc: 1 + 128 * (kc + 1)],
                                    qhT[HH[hh], 1 + g0: 1 + g0 + w],
                                    start=True, stop=True,
                                )
                            if w < 256:
                                for hh in range(2):
                                    nc.vector.memset(
                                        s2[hh][:, 256 * j + w: 256 * (j + 1)],
                                        0.0)
                        ma = _qwin(kca)[2]
                        mb = _qwin(kcb)[2]
                        for hh in range(2):
                            nc.scalar.activation(p2[hh][:], s2[hh][:], AF.Exp)
                            if ma == 0 and mb == 0:
                                nc.vector.tensor_tensor(
                                    p2[hh][:], p2[hh][:], mask2_sb[:], ALU.mult)
                            else:
                                wa = _qwin(kca)[1]
                                wb = _qwin(kcb)[1]
                                nc.vector.tensor_tensor(
                                    p2[hh][:, 0:wa], p2[hh][:, 0:wa],
                                    mask2_sb[:, ma: ma + wa], ALU.mult)
                                nc.vector.tensor_tensor(
                                    p2[hh][:, 256: 256 + wb],
                                    p2[hh][:, 256: 256 + wb],
                                    mask2_sb[:, mb: mb + wb], ALU.mult)
                        # PV accumulation (split at the PSUM bank boundary)
                        for hh in range(2):
                            h = 2 * hp + hh
                            for j, kc in enumerate((kca, kcb)):
                                g0, w, _ = _qwin(kc)
                                if g0 < 512 and g0 + w > 512:
                                    pieces = [(g0, 512 - g0), (512, g0 + w - 512)]
                                else:
                                    pieces = [(g0, w)]
                                off = 0
                                for pg0, pw in pieces:
                                    stop = (kc == 4 and pg0 + pw == 512) or \
                                           (kc == 7 and pg0 + pw == G)
                                    half = 1 if pg0 >= 512 else 0
                                    nc.tensor.matmul(
                                        pv2[hh][half][:, pg0 - 512 * half:
                                                       pg0 - 512 * half + pw],
                                        vh_panel[:, kc, h, :],
                                        p2[hh][:, 256 * j + off: 256 * j + off + pw],
                                        start=False, stop=stop,
                                        skip_group_check=True,
                                    )
                                    off += pw
                        fill(1)

                    for g in pending:
                        g()
                    pending = []

                    # ---- normalize + evacuate (z on PSUM partition DH,
                    # hoisted to SBUF partition 0 by the scalar engine) ----
                    zsb2 = []
                    for hh in range(2):
                        zsb = small.tile([1, G], F32, tag="zsb",
                                         name=f"zsb{hh}")
                        for j in range(2):
                            nc.scalar.copy(zsb[:, 512 * j: 512 * (j + 1)],
                                           pv2[hh][j][DH: DH + 1, :])
                        zsb2.append(zsb)
                    for hh in range(2):
                        rz = small.tile([1, G], F32, tag="rz")
                        nc.vector.reciprocal_approx_fast(rz[:], zsb2[hh][:])
                        rzb = bcast.tile([DH, G], F32, tag="rzb")
                        nc.gpsimd.partition_broadcast(rzb[:], rz[:])
                        if hh == 0:
                            for j in range(2):
                                nc.vector.tensor_tensor(
                                    attnout[HH[0], hp, 1 + 512 * j:
                                            1 + 512 * (j + 1)],
                                    pv2[0][j][0:DH, :],
                                    rzb[:, 512 * j: 512 * (j + 1)], ALU.mult)
                        else:
                            tmp = bcast.tile([DH, G], BF16, tag="tmp")
                            for j in range(2):
                                nc.vector.tensor_tensor(
                                    tmp[:, 512 * j: 512 * (j + 1)],
                                    pv2[1][j][0:DH, :],
                                    rzb[:, 512 * j: 512 * (j + 1)], ALU.mult)
                            nc.sync.dma_start(attnout[HH[1], hp, 1:NT],
                                              tmp[:])

                    # ---- BOS-query epilogue (pq2 already normalized) ----
                    for hh in range(2):
                        h = 2 * hp + hh
                        pvq = psA.tile([P, 512], F32, tag="ps")
                        nc.tensor.matmul(pvq[0:DH, 0:1],
                                         vbos65[0:1, h, 0:DH],
                                         pq2[0:1, hh, 0:1],
                                         start=True, stop=False,
                                         skip_group_check=True)
                        for c in range(KC):
                            nc.tensor.matmul(
                                pvq[0:DH, 0:1], vh_panel[:, c, h, 0:DH],
                                pq2[:, hh, 1 + c: 2 + c],
                                start=False, stop=(c == KC - 1),
                                skip_group_check=True,
                            )
                        if hh == 0:
                            nc.vector.tensor_copy(attnout[HH[0], hp, 0:1],
                                                  pvq[0:DH, 0:1])
                        else:
                            tmpb = small.tile([DH, 1], BF16, tag="tmpb")
                            nc.vector.tensor_copy(tmpb[:], pvq[0:DH, 0:1])
                            nc.sync.dma_start(attnout[HH[1], hp, 0:1], tmpb[:])

                # ---- output projection ----
                for mt in range(9):
                    t0 = 128 * mt
                    tw = 128 if mt < 8 else 1
                    for half in range(2):
                        acc = psA.tile([P, 512], F32, tag="ps")
                        for ct in range(KT):
                            nc.tensor.matmul(
                                acc[0:tw, :],
                                attnout[:, ct, t0: t0 + tw],
                                wo_c[ct][:, 512 * half: 512 * (half + 1)],
                                start=(ct == 0), stop=(ct == KT - 1),
                            )
                        ost = bcast.tile([P, 512], F32, tag="ost")
                        nc.vector.tensor_copy(ost[0:tw, :], acc[0:tw, :])
                        nc.sync.dma_start(
                            out[b, t0: t0 + tw, 512 * half: 512 * (half + 1)],
                            ost[0:tw, :],
                        )

    nc.compile()
    return nc


_NC = None
LAST_RESULT = None


def _get_nc():
    global _NC
    if _NC is None:
        _NC = build_nc()
    return _NC


def kernel(q, k, v, Wq, Wk, Wv, Wo, bo):
    bf16 = ml_dtypes.bfloat16
    qT = np.ascontiguousarray(np.asarray(q, np.float32).transpose(0, 2, 1)).astype(bf16)
    kT = np.ascontiguousarray(np.asarray(k, np.float32).transpose(0, 2, 1)).astype(bf16)
    vT = np.ascontiguousarray(np.asarray(v, np.float32).transpose(0, 2, 1)).astype(bf16)
    wq16 = (np.asarray(Wq, np.float32) * SCALE).astype(bf16)
    wk16 = np.asarray(Wk, np.float32).astype(bf16)
    wv16 = np.asarray(Wv, np.float32).astype(bf16)
    wo16 = np.asarray(Wo, np.float32).astype(bf16)

    nc = _get_nc()
    in_maps = []
    for c in range(NCORES):
        sl = slice(BPC * c, BPC * (c + 1))
        in_maps.append({
            "qt": np.ascontiguousarray(qT[sl]),
            "kt": np.ascontiguousarray(kT[sl]),
            "vt": np.ascontiguousarray(vT[sl]),
            "wq": wq16, "wk": wk16, "wv": wv16, "wo": wo16,
        })
    res = None
    if os.environ.get("BASS_KERNEL_TRACE"):
        try:
            res = run_bass_kernel_spmd(
                nc, in_maps, core_ids=list(range(NCORES)), trace=True)
        except Exception as e:  # fall back to the untraced path
            print(f"trace run failed ({e!r}); rerunning without trace")
            res = None
    if res is None:
        res = run_bass_kernel_spmd(nc, in_maps, core_ids=list(range(NCORES)))
    global LAST_RESULT
    LAST_RESULT = res
    out = np.concatenate([r["out"] for r in res.results], axis=0)
    out = out + np.asarray(bo, np.float32)[None, None, :]
    return out.astype(np.float32)


if __name__ == "__main__":
    rng = np.random.default_rng(0)
    ins = {
        "q": rng.standard_normal((B, NT, DIM), np.float32),
        "k": rng.standard_normal((B, NT, DIM), np.float32),
        "v": rng.standard_normal((B, NT, DIM), np.float32),
        "Wq": rng.standard_normal((DIM, INNER), np.float32) * DIM ** -0.5,
        "Wk": rng.standard_normal((DIM, INNER), np.float32) * DIM ** -0.5,
        "Wv": rng.standard_normal((DIM, INNER), np.float32) * DIM ** -0.5,
        "Wo": rng.standard_normal((INNER, DIM), np.float32) * INNER ** -0.5,
        "bo": np.zeros((DIM,), np.float32),
    }
    o = kernel(**ins)
    print(o.shape, o.dtype, np.abs(o).max())


# revision 14
# speedup vs baseline: 1.2377x; 1.2377x over previous
"""NearbyAttention Trainium2 kernel.

Full-input contract: kernel(**inputs) takes the unsharded numpy inputs of
nn_NearbyAttention (q,k,v: [16,1025,1024] f32; Wq/Wk/Wv/Wo: [1024,1024] f32;
bo: [1024] f32) and returns the full [16,1025,1024] f32 output.

Strategy: 8-way data parallel over the batch dim (2 batches per NeuronCore),
weights replicated, no collectives. Host pre-transposes activations to
[dim, tokens], folds the 1/sqrt(dh) scale into Wq, and casts to bf16.

Per batch on device:
  - v projection into per-key-chunk head panels with a trailing ones column
    (softmax denominator rides the PV accumulation for free)
  - q/k head-pair projections software-pipelined one head-pair ahead of the
    sparse nearby attention so the PE stream stays dense (HAM stays warm)
  - BOS-as-key scores for both heads of a pair via one masked [128,65]
    stationary matmul + batched exp; BOS-as-query softmax denominator via
    activation accum_out; epilogue folded into the hp loop
  - output projection back to [tokens, 1024] fp32
"""

import os

import numpy as np
import ml_dtypes

import concourse.bass as bass
import concourse.mybir as mybir
import concourse.tile as tile
from concourse import bacc
from concourse.bass_utils import run_bass_kernel_spmd

BF16 = mybir.dt.bfloat16
F32 = mybir.dt.float32
AF = mybir.ActivationFunctionType
ALU = mybir.AluOpType

B = 16              # full batch
BPC = 2             # batches per core
NCORES = 8
NT = 1025           # tokens (BOS + 32*32 grid)
G = 1024            # grid tokens
DIM = 1024
HEADS = 16
DH = 64
INNER = HEADS * DH  # 1024
P = 128
KT = 8              # contraction tiles of 128
SCALE = DH ** -0.5  # 0.125 (folded into Wq on host)
NEG = -30.0         # mask bias for the handful of memset'd cells

KC = 8              # key chunks of 128 grid tokens (4 patch rows each)

HH = (slice(0, 64), slice(64, 128))


def _mask_pattern() -> np.ndarray:
    """[128 key-local, 256 query-local] 0/1 bf16 mask in S^T orientation.

    Key chunk rows kr=0..3 (absolute 4kc+kr); query window rows
    rho=0..7 (absolute 4kc-2+rho). Unmasked iff |kr+2-rho|<=2 and
    |kcol-qcol|<=2.
    """
    kr = np.arange(128)[:, None] // 32
    kcol = np.arange(128)[:, None] % 32
    qr = np.arange(256)[None, :] // 32
    qcol = np.arange(256)[None, :] % 32
    m = (np.abs(kr + 2 - qr) <= 2) & (np.abs(kcol - qcol) <= 2)
    return m.astype(ml_dtypes.bfloat16)


def _qwin(kc: int) -> tuple[int, int, int]:
    """Grid-query window for key chunk kc: (grid_start, width, mask_col_off)."""
    if kc == 0:
        return 0, 192, 64
    if kc == KC - 1:
        return 128 * kc - 64, 192, 0
    return 128 * kc - 64, 256, 0


def build_nc():
    nc = bacc.Bacc("TRN2", target_bir_lowering=False, debug=False,
                   num_devices=NCORES)

    qt = nc.dram_tensor("qt", [BPC, DIM, NT], BF16, kind="ExternalInput")
    kt = nc.dram_tensor("kt", [BPC, DIM, NT], BF16, kind="ExternalInput")
    vt = nc.dram_tensor("vt", [BPC, DIM, NT], BF16, kind="ExternalInput")
    wq = nc.dram_tensor("wq", [DIM, INNER], BF16, kind="ExternalInput")
    wk = nc.dram_tensor("wk", [DIM, INNER], BF16, kind="ExternalInput")
    wv = nc.dram_tensor("wv", [DIM, INNER], BF16, kind="ExternalInput")
    wo = nc.dram_tensor("wo", [INNER, DIM], BF16, kind="ExternalInput")
    out = nc.dram_tensor("out", [BPC, NT, DIM], F32, kind="ExternalOutput")

    mask_np = _mask_pattern()  # [128, 256]
    mask2_np = np.concatenate([mask_np, mask_np], axis=1)  # [128, 512]
    mask_dram = nc.inline_tensor(mask2_np, name="mask2")

    with tile.TileContext(nc) as tc:
        with (
            tc.tile_pool(name="singles", bufs=1) as singles,
            tc.tile_pool(name="perbatch", bufs=1) as perbatch,
            tc.tile_pool(name="hppool", bufs=3) as hppool,
            tc.tile_pool(name="ppool", bufs=3) as ppool,
            tc.tile_pool(name="small", bufs=2) as small,
            tc.tile_pool(name="bcast", bufs=2) as bcast,
            tc.tile_pool(name="psA", bufs=4, space="PSUM") as psA,
            tc.tile_pool(name="psPV", bufs=4, space="PSUM") as psPV,
        ):
            # ---- persistent weights/constants, split per 128-row ktile ----
            wq_c = [singles.tile([P, INNER], BF16, tag=f"wq{c}", name=f"wq{c}")
                    for c in range(KT)]
            wk_c = [singles.tile([P, INNER], BF16, tag=f"wk{c}", name=f"wk{c}")
                    for c in range(KT)]
            wv_c = [singles.tile([P, INNER], BF16, tag=f"wv{c}", name=f"wv{c}")
                    for c in range(KT)]
            wo_c = [singles.tile([P, DIM], BF16, tag=f"wo{c}", name=f"wo{c}")
                    for c in range(KT)]
            mask2_sb = singles.tile([P, 512], BF16, tag="mask2")
            ones_sb = singles.tile([P, 1], BF16, tag="ones")
            nc.vector.memset(ones_sb[:], 1.0)

            for b in range(BPC):
                # ---- load transposed activations (per-ktile slices) ----
                vT = [perbatch.tile([P, NT], BF16, tag=f"vT{c}", name=f"vT{c}")
                      for c in range(KT)]
                qT = [perbatch.tile([P, NT], BF16, tag=f"qT{c}", name=f"qT{c}")
                      for c in range(KT)]
                kT = [perbatch.tile([P, NT], BF16, tag=f"kT{c}", name=f"kT{c}")
                      for c in range(KT)]
                for c in range(KT):
                    nc.sync.dma_start(vT[c][:], vt[b, 128 * c: 128 * (c + 1), :])
                    if b == 0:
                        nc.sync.dma_start(wv_c[c][:], wv[128 * c: 128 * (c + 1), :])
                for c in range(KT):
                    nc.sync.dma_start(qT[c][:], qt[b, 128 * c: 128 * (c + 1), :])
                    if b == 0:
                        nc.sync.dma_start(wq_c[c][:], wq[128 * c: 128 * (c + 1), :])
                for c in range(KT):
                    nc.sync.dma_start(kT[c][:], kt[b, 128 * c: 128 * (c + 1), :])
                    if b == 0:
                        nc.sync.dma_start(wk_c[c][:], wk[128 * c: 128 * (c + 1), :])
                if b == 0:
                    nc.sync.dma_start(mask2_sb[:], mask_dram[:])
                    for c in range(KT):
                        nc.sync.dma_start(wo_c[c][:], wo[128 * c: 128 * (c + 1), :])

                # ---- v projection into head panels ----
                # vh_panel[g, kc, h, DH] = 1 (softmax denominator trick, z
                # on PSUM partition DH); [.., 0:DH] = (v@Wv)[token g+1, h*64:]
                vh_panel = perbatch.tile([P, KC, HEADS, DH + 1], BF16, tag="vhp")
                vbos = perbatch.tile([1, HEADS, DH + 1], BF16, tag="vbos")
                vbos65 = perbatch.tile([DH + 1, HEADS, DH + 1], BF16,
                                       tag="vbos65")
                nc.vector.memset(vh_panel[:, :, :, DH], 1.0)
                nc.vector.memset(vbos[:, :, DH], 1.0)

                for mt in range(KC):  # grid token chunks
                    for half in range(2):
                        acc = psA.tile([P, 512], F32, tag="ps")
                        for c in range(KT):
                            nc.tensor.matmul(
                                acc[:],
                                vT[c][:, 1 + 128 * mt: 1 + 128 * (mt + 1)],
                                wv_c[c][:, 512 * half: 512 * (half + 1)],
                                start=(c == 0), stop=(c == KT - 1),
                            )
                        nc.vector.tensor_copy(
                            vh_panel[:, mt, 8 * half: 8 * (half + 1), 0:DH],
                            acc.rearrange("p (h d) -> p h d", d=DH),
                        )
                # BOS token of v
                for half in range(2):
                    acc = psA.tile([P, 512], F32, tag="ps")
                    for c in range(KT):
                        nc.tensor.matmul(
                            acc[0:1, :], vT[c][:, 0:1],
                            wv_c[c][:, 512 * half: 512 * (half + 1)],
                            start=(c == 0), stop=(c == KT - 1),
                        )
                    nc.vector.tensor_copy(
                        vbos[:, 8 * half: 8 * (half + 1), 0:DH],
                        acc[0:1].rearrange("p (h d) -> p h d", d=DH),
                    )
                # replicate the BOS v panel across partitions so the PV
                # start matmul's stationary can be based at partition 0
                # (head 0) or 64 (head 1) to match the pbos65 row
                nc.gpsimd.partition_broadcast(vbos65[:], vbos[0:1])

                attnout = perbatch.tile([P, KC, NT], BF16, tag="attnout")
                khB = perbatch.tile([P, DH + 1], BF16, tag="khB")
                nc.vector.memset(khB[:], 0.0)

                # ---- software-pipelined q/k head-pair projections ----
                def make_proj_groups(hp, qhT, khT):
                    """Six emitters: (dst, chunk) psA groups for head pair hp."""
                    groups = []
                    for dst, w_c, src in ((khT, wk_c, kT), (qhT, wq_c, qT)):
                        for nt0, ntw in ((0, 512), (512, 512), (1024, 1)):
                            def g(dst=dst, w_c=w_c, src=src, nt0=nt0, ntw=ntw):
                                acc = psA.tile([P, 512], F32, tag="ps")
                                for c in range(KT):
                                    nc.tensor.matmul(
                                        acc[:, 0:ntw],
                                        w_c[c][:, 128 * hp: 128 * (hp + 1)],
                                        src[c][:, nt0: nt0 + ntw],
                                        start=(c == 0), stop=(c == KT - 1),
                                    )
                                nc.vector.tensor_copy(
                                    dst[:, nt0: nt0 + ntw], acc[:, 0:ntw])
                            groups.append(g)
                    return groups

                hp_tiles = []
                for hp in range(KC):
                    qhT = hppool.tile([P, NT], BF16, tag="qhT",
                                      name=f"qhT{hp}")
                    khT = hppool.tile([P, NT], BF16, tag="khT",
                                      name=f"khT{hp}")
                    hp_tiles.append((qhT, khT))

                pending = list(make_proj_groups(0, *hp_tiles[0]))

                for hp in range(KC):
                    qhT, khT = hp_tiles[hp]
                    for g in pending:  # finish this hp's projections
                        g()
                    pending = (list(make_proj_groups(hp + 1, *hp_tiles[hp + 1]))
                               if hp < KC - 1 else [])

                    def fill(n):
                        for _ in range(min(n, len(pending))):
                            pending.pop(0)()

                    # ---- BOS-as-key scores: head 0 lands on partition 0,
                    # head 1 on partition 64 (legal matmul base partitions).
                    # khB's zero background persists across hp iterations;
                    # only the two live columns are rewritten.
                    nc.vector.tensor_copy(khB[HH[0], 0:1], khT[HH[0], 0:1])
                    nc.vector.tensor_copy(khB[HH[1], DH: DH + 1],
                                          khT[HH[1], 0:1])
                    pbos65 = hppool.tile([DH + 1, NT], BF16, tag="pbos65")
                    for nt0, ntw in ((0, 512), (512, 512), (1024, 1)):
                        acc = psA.tile([P, 512], F32, tag="ps")
                        nc.tensor.matmul(acc[0:DH + 1, 0:ntw], khB[:],
                                         qhT[:, nt0: nt0 + ntw],
                                         start=True, stop=True)
                        nc.scalar.activation(pbos65[:, nt0: nt0 + ntw],
                                             acc[0:DH + 1, 0:ntw], AF.Exp)
                    fill(2)

                    # ---- BOS-as-query scores; zq via exp accum_out ----
                    pq2 = hppool.tile([P, 2, 9], BF16, tag="pq2")
                    pqsum = small.tile([P, 2], F32, tag="pqsum")
                    sq2 = [psA.tile([P, 512], F32, tag="ps", name=f"sq2_{i}")
                           for i in range(2)]
                    for hh in range(2):
                        nc.vector.memset(sq2[hh][:, 0:1], NEG)
                    for hh in range(2):
                        nc.tensor.matmul(sq2[hh][0:1, 0:1], khT[HH[hh], 0:1],
                                         qhT[HH[hh], 0:1], start=True, stop=True)
                        for c in range(KC):
                            nc.tensor.matmul(
                                sq2[hh][:, 1 + c: 2 + c],
                                khT[HH[hh], 1 + 128 * c: 1 + 128 * (c + 1)],
                                qhT[HH[hh], 0:1], start=True, stop=True,
                            )
                    for hh in range(2):
                        nc.scalar.activation(pq2[:, hh, :], sq2[hh][:, 0:9],
                                             AF.Exp,
                                             accum_out=pqsum[:, hh: hh + 1])
                    pqb = small.tile([P, 2], BF16, tag="pqb")
                    nc.vector.tensor_copy(pqb[:], pqsum[:])
                    fill(2)

                    def emit_zq_chain(pqb=pqb):
                        # deferred so the tiny zq matmuls never block the
                        # dense projection stream at the PE queue head
                        zq_ps = psA.tile([P, 512], F32, tag="ps")
                        for hh in range(2):
                            nc.tensor.matmul(zq_ps[0:1, hh: hh + 1],
                                             ones_sb[:], pqb[:, hh: hh + 1],
                                             start=True, stop=True)
                        rzq2 = small.tile([1, 2], F32, tag="rzq2")
                        nc.vector.reciprocal_approx_fast(rzq2[:],
                                                         zq_ps[0:1, 0:2])
                        rzqb = small.tile([P, 2], F32, tag="rzqb")
                        nc.gpsimd.partition_broadcast(rzqb[:], rzq2[:])
                        for hh in range(2):
                            nc.vector.tensor_tensor(
                                pq2[:, hh, :], pq2[:, hh, :],
                                rzqb[:, hh: hh + 1].to_broadcast([P, 9]),
                                ALU.mult)

                    # ---- main nearby attention, both heads interleaved ----
                    pv2 = [[psPV.tile([DH + 1, 512], F32, tag="pv",
                                      name=f"pv{i}_{j}") for j in range(2)]
                           for i in range(2)]
                    for hh in range(2):
                        h = 2 * hp + hh
                        hb = DH * hh  # base partition 0 or 64
                        for j in range(2):
                            nc.tensor.matmul(
                                pv2[hh][j][:], vbos65[hb: hb + 1, h, :],
                                pbos65[hb: hb + 1,
                                       1 + 512 * j: 1 + 512 * (j + 1)],
                                start=True, stop=False, skip_group_check=True)

                    for kcp in range(4):  # pairs of key chunks
                        kca, kcb = 2 * kcp, 2 * kcp + 1
                        s2 = [psA.tile([P, 512], F32, tag="ps", name=f"s2_{i}")
                              for i in range(2)]
                        p2 = [ppool.tile([P, 512], BF16, tag="p", name=f"p2_{i}")
                              for i in range(2)]
                        # all four QK matmuls adjacent: h0/h1 use disjoint
                        # PE row groups and run concurrently
                        for j, kc in enumerate((kca, kcb)):
                            g0, w, _ = _qwin(kc)
                            for hh in range(2):
                                nc.tensor.matmul(
                                    s2[hh][:, 256 * j: 256 * j + w],
                                    khT[HH[hh], 1 + 128 * kc: 1 + 128 * (kc + 1)],
                                    qhT[HH[hh], 1 + g0: 1 + g0 + w],
                                    start=True, stop=True,
                                )
                        ma = _qwin(kca)[2]
                        mb = _qwin(kcb)[2]
                        for hh in range(2):
                            nc.scalar.activation(p2[hh][:], s2[hh][:], AF.Exp)
                            if ma == 0 and mb == 0:
                                nc.vector.tensor_tensor(
                                    p2[hh][:], p2[hh][:], mask2_sb[:], ALU.mult)
                            else:
                                wa = _qwin(kca)[1]
                                wb = _qwin(kcb)[1]
                                nc.vector.tensor_tensor(
                                    p2[hh][:, 0:wa], p2[hh][:, 0:wa],
                                    mask2_sb[:, ma: ma + wa], ALU.mult)
                                nc.vector.tensor_tensor(
                                    p2[hh][:, 256: 256 + wb],
                                    p2[hh][:, 256: 256 + wb],
                                    mask2_sb[:, mb: mb + wb], ALU.mult)
                        # PV accumulation (split at the PSUM bank boundary)
                        for hh in range(2):
                            h = 2 * hp + hh
                            for j, kc in enumerate((kca, kcb)):
                                g0, w, _ = _qwin(kc)
                                if g0 < 512 and g0 + w > 512:
                                    pieces = [(g0, 512 - g0), (512, g0 + w - 512)]
                                else:
                                    pieces = [(g0, w)]
                                off = 0
                                for pg0, pw in pieces:
                                    stop = (kc == 4 and pg0 + pw == 512) or \
                                           (kc == 7 and pg0 + pw == G)
                                    half = 1 if pg0 >= 512 else 0
                                    nc.tensor.matmul(
                                        pv2[hh][half][:, pg0 - 512 * half:
                                                       pg0 - 512 * half + pw],
                                        vh_panel[:, kc, h, :],
                                        p2[hh][:, 256 * j + off: 256 * j + off + pw],
                                        start=False, stop=stop,
                                        skip_group_check=True,
                                    )
                                    off += pw
                        if kcp == 0:
                            emit_zq_chain()
                        fill(1)

                    for g in pending:
                        g()
                    pending = []

                    # ---- normalize + evacuate (z on PSUM partition DH,
                    # hoisted to SBUF partition 0 by the scalar engine) ----
                    zsb2 = []
                    for hh in range(2):
                        zsb = small.tile([1, G], F32, tag="zsb",
                                         name=f"zsb{hh}")
                        for j in range(2):
                            nc.scalar.copy(zsb[:, 512 * j: 512 * (j + 1)],
                                           pv2[hh][j][DH: DH + 1, :])
                        zsb2.append(zsb)
                    for hh in range(2):
                        rz = small.tile([1, G], F32, tag="rz")
                        nc.vector.reciprocal_approx_fast(rz[:], zsb2[hh][:])
                        rzb = bcast.tile([DH, G], F32, tag="rzb")
                        nc.gpsimd.partition_broadcast(rzb[:], rz[:])
                        if hh == 0:
                            for j in range(2):
                                nc.vector.tensor_tensor(
                                    attnout[HH[0], hp, 1 + 512 * j:
                                            1 + 512 * (j + 1)],
                                    pv2[0][j][0:DH, :],
                                    rzb[:, 512 * j: 512 * (j + 1)], ALU.mult)
                        else:
                            tmp = bcast.tile([DH, G], BF16, tag="tmp")
                            for j in range(2):
                                nc.vector.tensor_tensor(
                                    tmp[:, 512 * j: 512 * (j + 1)],
                                    pv2[1][j][0:DH, :],
                                    rzb[:, 512 * j: 512 * (j + 1)], ALU.mult)
                            nc.sync.dma_start(attnout[HH[1], hp, 1:NT],
                                              tmp[:])

                    # ---- BOS-query epilogue (pq2 already normalized) ----
                    for hh in range(2):
                        h = 2 * hp + hh
                        pvq = psA.tile([P, 512], F32, tag="ps")
                        nc.tensor.matmul(pvq[0:DH, 0:1],
                                         vbos65[0:1, h, 0:DH],
                                         pq2[0:1, hh, 0:1],
                                         start=True, stop=False,
                                         skip_group_check=True)
                        for c in range(KC):
                            nc.tensor.matmul(
                                pvq[0:DH, 0:1], vh_panel[:, c, h, 0:DH],
                                pq2[:, hh, 1 + c: 2 + c],
                                start=False, stop=(c == KC - 1),
                                skip_group_check=True,
                            )
                        if hh == 0:
                            nc.vector.tensor_copy(attnout[HH[0], hp, 0:1],
                                                  pvq[0:DH, 0:1])
                        else:
                            tmpb = small.tile([DH, 1], BF16, tag="tmpb")
                            nc.vector.tensor_copy(tmpb[:], pvq[0:DH, 0:1])
                            nc.sync.dma_start(attnout[HH[1], hp, 0:1], tmpb[:])

                # ---- output projection ----
                for mt in range(9):
                    t0 = 128 * mt
                    tw = 128 if mt < 8 else 1
                    for half in range(2):
                        acc = psA.tile([P, 512], F32, tag="ps")
                        for ct in range(KT):
                            nc.tensor.matmul(
                                acc[0:tw, :],
                                attnout[:, ct, t0: t0 + tw],
                                wo_c[ct][:, 512 * half: 512 * (half + 1)],
                                start=(ct == 0), stop=(ct == KT - 1),
                            )
                        ost = bcast.tile([P, 512], F32, tag="ost")
                        nc.vector.tensor_copy(ost[0:tw, :], acc[0:tw, :])
                        nc.sync.dma_start(
                            out[b, t0: t0 + tw, 512 * half: 512 * (half + 1)],
                            ost[0:tw, :],
                        )

    nc.compile()
    return nc


_NC = None
LAST_RESULT = None


def _get_nc():
    global _NC
    if _NC is None:
        _NC = build_nc()
    return _NC


def kernel(q, k, v, Wq, Wk, Wv, Wo, bo):
    bf16 = ml_dtypes.bfloat16
    qT = np.ascontiguousarray(np.asarray(q, np.float32).transpose(0, 2, 1)).astype(bf16)
    kT = np.ascontiguousarray(np.asarray(k, np.float32).transpose(0, 2, 1)).astype(bf16)
    vT = np.ascontiguousarray(np.asarray(v, np.float32).transpose(0, 2, 1)).astype(bf16)
    wq16 = (np.asarray(Wq, np.float32) * SCALE).astype(bf16)
    wk16 = np.asarray(Wk, np.float32).astype(bf16)
    wv16 = np.asarray(Wv, np.float32).astype(bf16)
    wo16 = np.asarray(Wo, np.float32).astype(bf16)

    nc = _get_nc()
    in_maps = []
    for c in range(NCORES):
        sl = slice(BPC * c, BPC * (c + 1))
        in_maps.append({
            "qt": np.ascontiguousarray(qT[sl]),
            "kt": np.ascontiguousarray(kT[sl]),
            "vt": np.ascontiguousarray(vT[sl]),
            "wq": wq16, "wk": wk16, "wv": wv16, "wo": wo16,
        })
    res = None
    if os.environ.get("BASS_KERNEL_TRACE"):
        try:
            res = run_bass_kernel_spmd(
                nc, in_maps, core_ids=list(range(NCORES)), trace=True)
        except Exception as e:  # fall back to the untraced path
            print(f"trace run failed ({e!r}); rerunning without trace")
            res = None
    if res is None:
        res = run_bass_kernel_spmd(nc, in_maps, core_ids=list(range(NCORES)))
    global LAST_RESULT
    LAST_RESULT = res
    out = np.concatenate([r["out"] for r in res.results], axis=0)
    out = out + np.asarray(bo, np.float32)[None, None, :]
    return out.astype(np.float32)


if __name__ == "__main__":
    rng = np.random.default_rng(0)
    ins = {
        "q": rng.standard_normal((B, NT, DIM), np.float32),
        "k": rng.standard_normal((B, NT, DIM), np.float32),
        "v": rng.standard_normal((B, NT, DIM), np.float32),
        "Wq": rng.standard_normal((DIM, INNER), np.float32) * DIM ** -0.5,
        "Wk": rng.standard_normal((DIM, INNER), np.float32) * DIM ** -0.5,
        "Wv": rng.standard_normal((DIM, INNER), np.float32) * DIM ** -0.5,
        "Wo": rng.standard_normal((INNER, DIM), np.float32) * INNER ** -0.5,
        "bo": np.zeros((DIM,), np.float32),
    }
    o = kernel(**ins)
    print(o.shape, o.dtype, np.abs(o).max())


# revision 16
# speedup vs baseline: 1.3264x; 1.0716x over previous
"""NearbyAttention Trainium2 kernel.

Full-input contract: kernel(**inputs) takes the unsharded numpy inputs of
nn_NearbyAttention (q,k,v: [16,1025,1024] f32; Wq/Wk/Wv/Wo: [1024,1024] f32;
bo: [1024] f32) and returns the full [16,1025,1024] f32 output.

Strategy: 8-way data parallel over the batch dim (2 batches per NeuronCore),
weights replicated, no collectives. Host pre-transposes activations to
[dim, tokens], folds the 1/sqrt(dh) scale into Wq, and casts to bf16.

Per batch on device:
  - v projection into per-key-chunk head panels with a trailing ones column
    (softmax denominator rides the PV accumulation for free)
  - q/k head-pair projections software-pipelined one head-pair ahead of the
    sparse nearby attention so the PE stream stays dense (HAM stays warm)
  - BOS-as-key scores for both heads of a pair via one masked [128,65]
    stationary matmul + batched exp; BOS-as-query softmax denominator via
    activation accum_out; epilogue folded into the hp loop
  - output projection back to [tokens, 1024] fp32
"""

import os

import numpy as np
import ml_dtypes

import concourse.bass as bass
import concourse.mybir as mybir
import concourse.tile as tile
from concourse import bacc
from concourse.bass_utils import run_bass_kernel_spmd

BF16 = mybir.dt.bfloat16
F32 = mybir.dt.float32
AF = mybir.ActivationFunctionType
ALU = mybir.AluOpType

B = 16              # full batch
BPC = 2             # batches per core
NCORES = 8
NT = 1025           # tokens (BOS + 32*32 grid)
G = 1024            # grid tokens
DIM = 1024
HEADS = 16
DH = 64
INNER = HEADS * DH  # 1024
P = 128
KT = 8              # contraction tiles of 128
SCALE = DH ** -0.5  # 0.125 (folded into Wq on host)
NEG = -30.0         # mask bias for the handful of memset'd cells

KC = 8              # key chunks of 128 grid tokens (4 patch rows each)

HH = (slice(0, 64), slice(64, 128))


def _mask_pattern() -> np.ndarray:
    """[128 key-local, 256 query-local] 0/1 bf16 mask in S^T orientation.

    Key chunk rows kr=0..3 (absolute 4kc+kr); query window rows
    rho=0..7 (absolute 4kc-2+rho). Unmasked iff |kr+2-rho|<=2 and
    |kcol-qcol|<=2.
    """
    kr = np.arange(128)[:, None] // 32
    kcol = np.arange(128)[:, None] % 32
    qr = np.arange(256)[None, :] // 32
    qcol = np.arange(256)[None, :] % 32
    m = (np.abs(kr + 2 - qr) <= 2) & (np.abs(kcol - qcol) <= 2)
    return m.astype(ml_dtypes.bfloat16)


def _qwin(kc: int) -> tuple[int, int, int]:
    """Grid-query window for key chunk kc: (grid_start, width, mask_col_off)."""
    if kc == 0:
        return 0, 192, 64
    if kc == KC - 1:
        return 128 * kc - 64, 192, 0
    return 128 * kc - 64, 256, 0


def build_nc():
    nc = bacc.Bacc("TRN2", target_bir_lowering=False, debug=False,
                   num_devices=NCORES)

    qt = nc.dram_tensor("qt", [BPC, DIM, NT], BF16, kind="ExternalInput")
    kt = nc.dram_tensor("kt", [BPC, DIM, NT], BF16, kind="ExternalInput")
    vt = nc.dram_tensor("vt", [BPC, DIM, NT], BF16, kind="ExternalInput")
    wq = nc.dram_tensor("wq", [DIM, INNER], BF16, kind="ExternalInput")
    wk = nc.dram_tensor("wk", [DIM, INNER], BF16, kind="ExternalInput")
    wv = nc.dram_tensor("wv", [DIM, INNER], BF16, kind="ExternalInput")
    wo = nc.dram_tensor("wo", [INNER, DIM], BF16, kind="ExternalInput")
    out = nc.dram_tensor("out", [BPC, NT, DIM], F32, kind="ExternalOutput")

    mask_np = _mask_pattern()  # [128, 256]
    mask2_np = np.concatenate([mask_np, mask_np], axis=1)  # [128, 512]
    mask_dram = nc.inline_tensor(mask2_np, name="mask2")

    with tile.TileContext(nc) as tc:
        with (
            tc.tile_pool(name="singles", bufs=1) as singles,
            tc.tile_pool(name="perbatch", bufs=1) as perbatch,
            tc.tile_pool(name="hppool", bufs=3) as hppool,
            tc.tile_pool(name="ppool", bufs=3) as ppool,
            tc.tile_pool(name="small", bufs=2) as small,
            tc.tile_pool(name="bcast", bufs=2) as bcast,
            tc.tile_pool(name="psA", bufs=4, space="PSUM") as psA,
            tc.tile_pool(name="psPV", bufs=4, space="PSUM") as psPV,
        ):
            # ---- persistent weights/constants, split per 128-row ktile ----
            wq_c = [singles.tile([P, INNER], BF16, tag=f"wq{c}", name=f"wq{c}")
                    for c in range(KT)]
            wk_c = [singles.tile([P, INNER], BF16, tag=f"wk{c}", name=f"wk{c}")
                    for c in range(KT)]
            wv_c = [singles.tile([P, INNER], BF16, tag=f"wv{c}", name=f"wv{c}")
                    for c in range(KT)]
            wo_c = [singles.tile([P, DIM], BF16, tag=f"wo{c}", name=f"wo{c}")
                    for c in range(KT)]
            mask2_sb = singles.tile([P, 512], BF16, tag="mask2")
            ones_sb = singles.tile([P, 1], BF16, tag="ones")
            nc.vector.memset(ones_sb[:], 1.0)

            # value panels, layout [ones | 31 zeros | 64 head dims] so that
            # in the PV accumulation z lands on PSUM partition 0 (for the
            # direct DVE reciprocal) and the head dims land on partitions
            # 32..95 (32-aligned for legal PSUM access). The ones/zero
            # bands are batch-invariant: written once, reused.
            VD = 32            # first dim column
            VW = VD + DH       # panel width 96
            vh_panel = singles.tile([P, KC, HEADS, VW], BF16, tag="vhp")
            vbos = singles.tile([1, HEADS, VW], BF16, tag="vbos")
            vbos65 = singles.tile([DH + 1, HEADS, VW], BF16, tag="vbos65")
            nc.vector.memset(vh_panel[:, :, :, 0:VD], 0.0)
            nc.vector.memset(vh_panel[:, :, :, 0], 1.0)
            nc.vector.memset(vbos[:, :, 0:VD], 0.0)
            nc.vector.memset(vbos[:, :, 0], 1.0)

            for b in range(BPC):
                # ---- load transposed activations (per-ktile slices) ----
                vT = [perbatch.tile([P, NT], BF16, tag=f"vT{c}", name=f"vT{c}")
                      for c in range(KT)]
                qT = [perbatch.tile([P, NT], BF16, tag=f"qT{c}", name=f"qT{c}")
                      for c in range(KT)]
                kT = [perbatch.tile([P, NT], BF16, tag=f"kT{c}", name=f"kT{c}")
                      for c in range(KT)]
                for c in range(KT):
                    nc.sync.dma_start(vT[c][:], vt[b, 128 * c: 128 * (c + 1), :])
                    if b == 0:
                        nc.sync.dma_start(wv_c[c][:], wv[128 * c: 128 * (c + 1), :])
                for c in range(KT):
                    nc.sync.dma_start(qT[c][:], qt[b, 128 * c: 128 * (c + 1), :])
                    if b == 0:
                        nc.sync.dma_start(wq_c[c][:], wq[128 * c: 128 * (c + 1), :])
                for c in range(KT):
                    nc.sync.dma_start(kT[c][:], kt[b, 128 * c: 128 * (c + 1), :])
                    if b == 0:
                        nc.sync.dma_start(wk_c[c][:], wk[128 * c: 128 * (c + 1), :])
                if b == 0:
                    nc.sync.dma_start(mask2_sb[:], mask_dram[:])
                    for c in range(KT):
                        nc.sync.dma_start(wo_c[c][:], wo[128 * c: 128 * (c + 1), :])

                # ---- v projection into head panels ----
                for mt in range(KC):  # grid token chunks
                    for half in range(2):
                        acc = psA.tile([P, 512], F32, tag="ps")
                        for c in range(KT):
                            nc.tensor.matmul(
                                acc[:],
                                vT[c][:, 1 + 128 * mt: 1 + 128 * (mt + 1)],
                                wv_c[c][:, 512 * half: 512 * (half + 1)],
                                start=(c == 0), stop=(c == KT - 1),
                            )
                        nc.vector.tensor_copy(
                            vh_panel[:, mt, 8 * half: 8 * (half + 1), VD:VW],
                            acc.rearrange("p (h d) -> p h d", d=DH),
                        )
                # BOS token of v
                for half in range(2):
                    acc = psA.tile([P, 512], F32, tag="ps")
                    for c in range(KT):
                        nc.tensor.matmul(
                            acc[0:1, :], vT[c][:, 0:1],
                            wv_c[c][:, 512 * half: 512 * (half + 1)],
                            start=(c == 0), stop=(c == KT - 1),
                        )
                    nc.vector.tensor_copy(
                        vbos[:, 8 * half: 8 * (half + 1), VD:VW],
                        acc[0:1].rearrange("p (h d) -> p h d", d=DH),
                    )
                # replicate the BOS v panel across partitions so the PV
                # start matmul's stationary can be based at partition 0
                # (head 0) or 64 (head 1) to match the pbos65 row
                nc.gpsimd.partition_broadcast(vbos65[:], vbos[0:1])

                attnout = perbatch.tile([P, KC, NT], BF16, tag="attnout")
                khB = perbatch.tile([P, DH + 1], BF16, tag="khB")
                nc.vector.memset(khB[:], 0.0)

                # ---- software-pipelined q/k head-pair projections ----
                def make_proj_groups(hp, qhT, khT):
                    """Six emitters: (dst, chunk) psA groups for head pair hp."""
                    groups = []
                    for dst, w_c, src in ((khT, wk_c, kT), (qhT, wq_c, qT)):
                        for nt0, ntw in ((0, 512), (512, 512), (1024, 1)):
                            def g(dst=dst, w_c=w_c, src=src, nt0=nt0, ntw=ntw):
                                acc = psA.tile([P, 512], F32, tag="ps")
                                for c in range(KT):
                                    nc.tensor.matmul(
                                        acc[:, 0:ntw],
                                        w_c[c][:, 128 * hp: 128 * (hp + 1)],
                                        src[c][:, nt0: nt0 + ntw],
                                        start=(c == 0), stop=(c == KT - 1),
                                    )
                                nc.vector.tensor_copy(
                                    dst[:, nt0: nt0 + ntw], acc[:, 0:ntw])
                            groups.append(g)
                    return groups

                hp_tiles = []
                for hp in range(KC):
                    qhT = hppool.tile([P, NT], BF16, tag="qhT",
                                      name=f"qhT{hp}")
                    khT = hppool.tile([P, NT], BF16, tag="khT",
                                      name=f"khT{hp}")
                    hp_tiles.append((qhT, khT))

                pending = list(make_proj_groups(0, *hp_tiles[0]))

                for hp in range(KC):
                    qhT, khT = hp_tiles[hp]
                    for g in pending:  # finish this hp's projections
                        g()
                    pending = (list(make_proj_groups(hp + 1, *hp_tiles[hp + 1]))
                               if hp < KC - 1 else [])

                    def fill(n):
                        for _ in range(min(n, len(pending))):
                            pending.pop(0)()

                    # ---- BOS-as-key scores: head 0 lands on partition 0,
                    # head 1 on partition 64 (legal matmul base partitions).
                    # khB's zero background persists across hp iterations;
                    # only the two live columns are rewritten.
                    nc.vector.tensor_copy(khB[HH[0], 0:1], khT[HH[0], 0:1])
                    nc.vector.tensor_copy(khB[HH[1], DH: DH + 1],
                                          khT[HH[1], 0:1])
                    pbos65 = hppool.tile([DH + 1, NT], BF16, tag="pbos65")
                    for nt0, ntw in ((0, 512), (512, 512), (1024, 1)):
                        acc = psA.tile([P, 512], F32, tag="ps")
                        nc.tensor.matmul(acc[0:DH + 1, 0:ntw], khB[:],
                                         qhT[:, nt0: nt0 + ntw],
                                         start=True, stop=True)
                        nc.scalar.activation(pbos65[:, nt0: nt0 + ntw],
                                             acc[0:DH + 1, 0:ntw], AF.Exp)
                    fill(2)

                    # ---- BOS-as-query scores; zq via exp accum_out ----
                    pq2 = hppool.tile([P, 2, 9], BF16, tag="pq2")
                    pqsum = small.tile([P, 2], F32, tag="pqsum")
                    sq2 = [psA.tile([P, 512], F32, tag="ps", name=f"sq2_{i}")
                           for i in range(2)]
                    for hh in range(2):
                        nc.vector.memset(sq2[hh][:, 0:1], NEG)
                    for hh in range(2):
                        nc.tensor.matmul(sq2[hh][0:1, 0:1], khT[HH[hh], 0:1],
                                         qhT[HH[hh], 0:1], start=True, stop=True)
                        for c in range(KC):
                            nc.tensor.matmul(
                                sq2[hh][:, 1 + c: 2 + c],
                                khT[HH[hh], 1 + 128 * c: 1 + 128 * (c + 1)],
                                qhT[HH[hh], 0:1], start=True, stop=True,
                            )
                    for hh in range(2):
                        nc.scalar.activation(pq2[:, hh, :], sq2[hh][:, 0:9],
                                             AF.Exp,
                                             accum_out=pqsum[:, hh: hh + 1])
                    pqb = small.tile([P, 2], BF16, tag="pqb")
                    nc.vector.tensor_copy(pqb[:], pqsum[:])
                    fill(2)

                    def emit_zq_chain(pqb=pqb):
                        # deferred so the tiny zq matmuls never block the
                        # dense projection stream at the PE queue head
                        zq_ps = psA.tile([P, 512], F32, tag="ps")
                        for hh in range(2):
                            nc.tensor.matmul(zq_ps[0:1, hh: hh + 1],
                                             ones_sb[:], pqb[:, hh: hh + 1],
                                             start=True, stop=True)
                        rzq2 = small.tile([1, 2], F32, tag="rzq2")
                        nc.vector.reciprocal_approx_fast(rzq2[:],
                                                         zq_ps[0:1, 0:2])
                        rzqb = small.tile([P, 2], F32, tag="rzqb")
                        nc.gpsimd.partition_broadcast(rzqb[:], rzq2[:])
                        for hh in range(2):
                            nc.vector.tensor_tensor(
                                pq2[:, hh, :], pq2[:, hh, :],
                                rzqb[:, hh: hh + 1].to_broadcast([P, 9]),
                                ALU.mult)

                    # ---- main nearby attention, both heads interleaved ----
                    pv2 = [[psPV.tile([VW, 512], F32, tag="pv",
                                      name=f"pv{i}_{j}") for j in range(2)]
                           for i in range(2)]

                    for kcp in range(4):  # pairs of key chunks
                        kca, kcb = 2 * kcp, 2 * kcp + 1
                        s2 = [psA.tile([P, 512], F32, tag="ps", name=f"s2_{i}")
                              for i in range(2)]
                        p2 = [ppool.tile([P, 512], BF16, tag="p", name=f"p2_{i}")
                              for i in range(2)]
                        # all four QK matmuls adjacent: h0/h1 use disjoint
                        # PE row groups and run concurrently
                        for j, kc in enumerate((kca, kcb)):
                            g0, w, _ = _qwin(kc)
                            for hh in range(2):
                                nc.tensor.matmul(
                                    s2[hh][:, 256 * j: 256 * j + w],
                                    khT[HH[hh], 1 + 128 * kc: 1 + 128 * (kc + 1)],
                                    qhT[HH[hh], 1 + g0: 1 + g0 + w],
                                    start=True, stop=True,
                                )
                        if kcp == 0:
                            # PV start matmuls sit here (not earlier) so the
                            # in-order PE queue never blocks on the BOS-key
                            # exp still draining through the scalar engine
                            for hh in range(2):
                                h = 2 * hp + hh
                                hb = DH * hh  # base partition 0 or 64
                                for j in range(2):
                                    nc.tensor.matmul(
                                        pv2[hh][j][:],
                                        vbos65[hb: hb + 1, h, :],
                                        pbos65[hb: hb + 1,
                                               1 + 512 * j: 1 + 512 * (j + 1)],
                                        start=True, stop=False,
                                        skip_group_check=True)
                        ma = _qwin(kca)[2]
                        mb = _qwin(kcb)[2]
                        for hh in range(2):
                            nc.scalar.activation(p2[hh][:], s2[hh][:], AF.Exp)
                            if ma == 0 and mb == 0:
                                nc.vector.tensor_tensor(
                                    p2[hh][:], p2[hh][:], mask2_sb[:], ALU.mult)
                            else:
                                wa = _qwin(kca)[1]
                                wb = _qwin(kcb)[1]
                                nc.vector.tensor_tensor(
                                    p2[hh][:, 0:wa], p2[hh][:, 0:wa],
                                    mask2_sb[:, ma: ma + wa], ALU.mult)
                                nc.vector.tensor_tensor(
                                    p2[hh][:, 256: 256 + wb],
                                    p2[hh][:, 256: 256 + wb],
                                    mask2_sb[:, mb: mb + wb], ALU.mult)
                        # PV accumulation (split at the PSUM bank boundary)
                        for hh in range(2):
                            h = 2 * hp + hh
                            for j, kc in enumerate((kca, kcb)):
                                g0, w, _ = _qwin(kc)
                                if g0 < 512 and g0 + w > 512:
                                    pieces = [(g0, 512 - g0), (512, g0 + w - 512)]
                                else:
                                    pieces = [(g0, w)]
                                off = 0
                                for pg0, pw in pieces:
                                    stop = (kc == 4 and pg0 + pw == 512) or \
                                           (kc == 7 and pg0 + pw == G)
                                    half = 1 if pg0 >= 512 else 0
                                    nc.tensor.matmul(
                                        pv2[hh][half][:, pg0 - 512 * half:
                                                       pg0 - 512 * half + pw],
                                        vh_panel[:, kc, h, :],
                                        p2[hh][:, 256 * j + off: 256 * j + off + pw],
                                        start=False, stop=stop,
                                        skip_group_check=True,
                                    )
                                    off += pw
                        if kcp == 0:
                            emit_zq_chain()
                        fill(1)

                    for g in pending:
                        g()
                    pending = []

                    # ---- normalize + evacuate (z on PSUM partition 0;
                    # reciprocal reads it straight out of PSUM) ----
                    for hh in range(2):
                        rz = small.tile([1, G], F32, tag="rz")
                        for j in range(2):
                            nc.vector.reciprocal_approx_fast(
                                rz[:, 512 * j: 512 * (j + 1)],
                                pv2[hh][j][0:1, :])
                        rzb = bcast.tile([VW, G], F32, tag="rzb")
                        nc.gpsimd.partition_broadcast(rzb[:], rz[:])
                        tmp = bcast.tile([VW, G], BF16, tag="tmp")
                        for j in range(2):
                            # PSUM reads from partition 32 are capped at 32
                            # partitions, so evacuate in two 32-row blocks
                            for p0 in (VD, VD + 32):
                                nc.vector.tensor_tensor(
                                    tmp[p0: p0 + 32, 512 * j: 512 * (j + 1)],
                                    pv2[hh][j][p0: p0 + 32, :],
                                    rzb[p0: p0 + 32, 512 * j: 512 * (j + 1)],
                                    ALU.mult)
                        nc.sync.dma_start(attnout[HH[hh], hp, 1:NT],
                                          tmp[VD:VW, :])

                    # ---- BOS-query epilogue (pq2 already normalized) ----
                    for hh in range(2):
                        h = 2 * hp + hh
                        pvq = psA.tile([P, 512], F32, tag="ps")
                        nc.tensor.matmul(pvq[0:DH, 0:1],
                                         vbos65[0:1, h, VD:VW],
                                         pq2[0:1, hh, 0:1],
                                         start=True, stop=False,
                                         skip_group_check=True)
                        for c in range(KC):
                            nc.tensor.matmul(
                                pvq[0:DH, 0:1], vh_panel[:, c, h, VD:VW],
                                pq2[:, hh, 1 + c: 2 + c],
                                start=False, stop=(c == KC - 1),
                                skip_group_check=True,
                            )
                        if hh == 0:
                            nc.vector.tensor_copy(attnout[HH[0], hp, 0:1],
                                                  pvq[0:DH, 0:1])
                        else:
                            tmpb = small.tile([DH, 1], BF16, tag="tmpb")
                            nc.vector.tensor_copy(tmpb[:], pvq[0:DH, 0:1])
                            nc.sync.dma_start(attnout[HH[1], hp, 0:1], tmpb[:])

                # ---- output projection ----
                for mt in range(9):
                    t0 = 128 * mt
                    tw = 128 if mt < 8 else 1
                    for half in range(2):
                        acc = psA.tile([P, 512], F32, tag="ps")
                        for ct in range(KT):
                            nc.tensor.matmul(
                                acc[0:tw, :],
                                attnout[:, ct, t0: t0 + tw],
                                wo_c[ct][:, 512 * half: 512 * (half + 1)],
                                start=(ct == 0), stop=(ct == KT - 1),
                            )
                        ost = bcast.tile([P, 512], F32, tag="ost")
                        nc.vector.tensor_copy(ost[0:tw, :], acc[0:tw, :])
                        nc.sync.dma_start(
                            out[b, t0: t0 + tw, 512 * half: 512 * (half + 1)],
                            ost[0:tw, :],
                        )

    nc.compile()
    return nc


_NC = None
LAST_RESULT = None


def _get_nc():
    global _NC
    if _NC is None:
        _NC = build_nc()
    return _NC


def kernel(q, k, v, Wq, Wk, Wv, Wo, bo):
    bf16 = ml_dtypes.bfloat16
    qT = np.ascontiguousarray(np.asarray(q, np.float32).transpose(0, 2, 1)).astype(bf16)
    kT = np.ascontiguousarray(np.asarray(k, np.float32).transpose(0, 2, 1)).astype(bf16)
    vT = np.ascontiguousarray(np.asarray(v, np.float32).transpose(0, 2, 1)).astype(bf16)
    wq16 = (np.asarray(Wq, np.float32) * SCALE).astype(bf16)
    wk16 = np.asarray(Wk, np.float32).astype(bf16)
    wv16 = np.asarray(Wv, np.float32).astype(bf16)
    wo16 = np.asarray(Wo, np.float32).astype(bf16)

    nc = _get_nc()
    in_maps = []
    for c in range(NCORES):
        sl = slice(BPC * c, BPC * (c + 1))
        in_maps.append({
            "qt": np.ascontiguousarray(qT[sl]),
            "kt": np.ascontiguousarray(kT[sl]),
            "vt": np.ascontiguousarray(vT[sl]),
            "wq": wq16, "wk": wk16, "wv": wv16, "wo": wo16,
        })
    res = None
    if os.environ.get("BASS_KERNEL_TRACE"):
        try:
            res = run_bass_kernel_spmd(
                nc, in_maps, core_ids=list(range(NCORES)), trace=True)
        except Exception as e:  # fall back to the untraced path
            print(f"trace run failed ({e!r}); rerunning without trace")
            res = None
    if res is None:
        res = run_bass_kernel_spmd(nc, in_maps, core_ids=list(range(NCORES)))
    global LAST_RESULT
    LAST_RESULT = res
    out = np.concatenate([r["out"] for r in res.results], axis=0)
    out = out + np.asarray(bo, np.float32)[None, None, :]
    return out.astype(np.float32)


if __name__ == "__main__":
    rng = np.random.default_rng(0)
    ins = {
        "q": rng.standard_normal((B, NT, DIM), np.float32),
        "k": rng.standard_normal((B, NT, DIM), np.float32),
        "v": rng.standard_normal((B, NT, DIM), np.float32),
        "Wq": rng.standard_normal((DIM, INNER), np.float32) * DIM ** -0.5,
        "Wk": rng.standard_normal((DIM, INNER), np.float32) * DIM ** -0.5,
        "Wv": rng.standard_normal((DIM, INNER), np.float32) * DIM ** -0.5,
        "Wo": rng.standard_normal((INNER, DIM), np.float32) * INNER ** -0.5,
        "bo": np.zeros((DIM,), np.float32),
    }
    o = kernel(**ins)
    print(o.shape, o.dtype, np.abs(o).max())
